# revision 40
# baseline (speedup 1.0000x reference)
"""DockingAwareAttention on 8 TRN2 NeuronCores — v2 (fp8 + engine-split exp).

Sharding: data-parallel over batch (2) x tensor-parallel over heads (4 groups
of 4 heads). Core c handles batch c//4 and head group c%4. Each core computes
its heads' softmax-path contribution through the row-sharded out-projection;
the host sums the 4 partials per batch.

Key structure (per core):
  - The post-softmax docking blend (1-a)*attn + a*ds makes the docking
    contribution to ctx independent of the query position, so it collapses
    to one exact rank-1 row per batch, computed on the HOST in float64:
        row = [a*(ds@x)@v_w + ((1-a) + a*sum(ds))*v_b] @ o_w + o_b
    The device computes only the (1-a)*softmax(QK^T)V @ o_w part.
  - Q/K/V projections: fp8 DoubleRow matmuls (K packed 2x along contraction),
    Q/K emitted to bf16 (scores run at bf16 rate anyway), V to fp8.
  - scores: per head-pair, two K=64 matmuls row-packed into PE row groups
    (0,0)/(64,0) -> concurrent execution.
  - exp over S^2 is the elementwise bottleneck: split across ScalarE
    (true exp -> fp8, scale 1/32) and VectorE (Schraudolph: byte =
    round(1.4427*s + 16) saturating-uint8, bitcast to fp8e4m3 ==
    exp(s/8)/32 within ~8%). The scale cancels in the softmax ratio.
  - PV: fp8 DoubleRow with a fused ones-column (M=65) giving row-sums r';
    1/r' via the int32 bit-trick reciprocal (C - bits(x)), ~5% err.
  - ctx scaled by 64 and o_w by 64 to keep fp8 in normal range; host
    divides the summed partials by 4096.

Known wall: the HAM micro-idle throttler. With DR the PE finishes each
kp's matmuls faster than the exp engines produce E (~0.9 vs 1.15us), the
recurring micro-idle makes HAM oscillate and ~50% of the kernel runs at
K=4/8. Without DR the PE is tensor-bound (no idle, no throttle) but needs
~1.8x the matmul cycles. Both land at ~228us; DR-everywhere is the
default. Breaking this needs either PSUM for PE filler work (none left:
big 3x4KB + psc 2x2KB = 16KB exactly) or a third exp-capable engine
(gpsimd has no PSUM port).
"""

import os
import sys

for _p in ("/opt/trn_rl_repo", "/root/.axon_site/_ro/trn_rl_repo", "/root/.axon_site"):
    if os.path.isdir(_p) and _p not in sys.path:
        sys.path.append(_p)

import numpy as np
import ml_dtypes

import concourse.bass as bass
import concourse.bacc as bacc
import concourse.mybir as mybir
from concourse import tile
from concourse import bass_utils

D = 1024          # model dim
S = 2048          # sequence length
B = 2             # batch
HL = 4            # heads per core
HD = 64           # head dim
EL = 256          # per-core projected dims
NQ = 512          # q tile
KC = 16           # s-chunks of 128
KP = 8            # s-chunk pairs (DoubleRow)
DP = 4            # d-chunk pairs
VW = 68           # per-head slot width in Vp (64 dims + ones col at 64, pad)

WQK_S = 32.0      # host scale on q_w/k_w (descaled in the psum copy)
WV_S = 32.0       # host scale on v_w
WO_S = 64.0       # host scale on o_w
CTX_S = 64.0      # device ctx scale
OUT_DIV = CTX_S * WO_S  # host divides partials by this

# E = exp(z)/32: the dataset's max logit is ~8.2 (heavier tail than iid
# N(0,1) suggests), so 1/32 keeps the ScalarE exp below TRN fp8e4's 240 max
# (e^{8.35-3.47}=132) and the Schraudolph byte below the 0x7F NaN (max ~113).
ECLAMP = 5 * np.log(2.0)               # 3.4657
SCHRAU_A = 8 * np.log2(np.e) * 0.125   # = 1.442695
SCHRAU_B = 56.0 - 40.0                 # exponent bias 56, minus 5 octaves (1/32)

f32 = mybir.dt.float32
bf16 = mybir.dt.bfloat16
f8 = mybir.dt.float8e4
u8 = mybir.dt.uint8
i32 = mybir.dt.int32
MULT = mybir.AluOpType.mult
ADD = mybir.AluOpType.add
BNOT = mybir.AluOpType.bitwise_not
EXP = mybir.ActivationFunctionType.Exp
IDENT = mybir.ActivationFunctionType.Identity
DR = mybir.MatmulPerfMode.DoubleRow

# bits(1/x) ~= RECIP_C - bits(x) (Blinn/Schraudolph); done as (x * -1) + C
# in int32 (no overflow for positive normal x). ~5% max rel err -- plenty
# for the softmax path.
RECIP_C = 0x7EF311C3

_CACHE = {}

def _install_ntff_hook_shim():
    """The image's antenv lacks axon_hooks, which silently disables NTFF
    profiling (trace=True). Provide the module and install the hook so
    BASS_TRACE=1 works."""
    import types

    if "antenv.axon_hooks" in sys.modules:
        return
    mod = types.ModuleType("antenv.axon_hooks")
    mod._hook = None

    def set_axon_ntff_profile_hook(h):
        mod._hook = h

    def get_axon_ntff_profile_hook():
        return mod._hook

    mod.set_axon_ntff_profile_hook = set_axon_ntff_profile_hook
    mod.get_axon_ntff_profile_hook = get_axon_ntff_profile_hook
    sys.modules["antenv.axon_hooks"] = mod
    try:
        import antenv

        antenv.axon_hooks = mod
        from trn_agent_boot.trn_boot import _ntff_profile_via_ctypes

        hook = _ntff_profile_via_ctypes("/opt/axon/libaxon_pjrt.so")
        if hook is not None:
            mod._hook = hook
    except Exception:
        pass


def _exp_engine(qt, pc, kp, par):
    """Which engine computes exp for this scores tile. par0->ScalarE,
    par1->VectorE, EXCEPT the last kp of each pc goes entirely to ScalarE so
    the VectorE FIFO is clear for the psc drain chain (reciprocal + ctx
    scale) at the pc transition -- otherwise the next pc's PV accumulators
    stall ~1.7us behind queued exps."""
    return "act" if (par == 0 or kp == KP - 1) else "dve"


def _build(alpha: float):
    sv = CTX_S * (1.0 - alpha) / WV_S  # Vp copy scale: 64*(1-a)*V_true
    # DoubleRow site selection: the per-NC throttler is energy-driven (full
    # DR: 121us at half clock; no DR: 27us) -- a partial-DR mix can stay
    # under the trip point while keeping most of the cycle savings.
    drmode = os.environ.get("BASS_DR", "all")  # all | pv | none
    nodr = drmode == "none"

    nc = bacc.Bacc(
        "TRN2",
        target_bir_lowering=False,
        debug=False,
        enable_asserts=False,
        num_devices=8,
    )

    xt_d = nc.dram_tensor("xt", (DP * 128, 2 * S), f8, kind="ExternalInput")
    wq_d = nc.dram_tensor("wq", (DP * 128, 2 * EL), f8, kind="ExternalInput")
    wk_d = nc.dram_tensor("wk", (DP * 128, 2 * EL), f8, kind="ExternalInput")
    wv_d = nc.dram_tensor("wv", (DP * 128, 2 * EL), f8, kind="ExternalInput")
    wo_d = nc.dram_tensor("wo", (128, 2 * D), f8, kind="ExternalInput")
    qb_d = nc.dram_tensor("qb", (128, 2), f32, kind="ExternalInput")
    kb_d = nc.dram_tensor("kb", (128, 2), f32, kind="ExternalInput")
    out_d = nc.dram_tensor("out", (S, D), bf16, kind="ExternalOutput")

    debug = os.environ.get("BASSDBG", "0") == "1"
    if debug:
        dvp_d = nc.dram_tensor("dvp", (128, 2 * HL * VW), u8, kind="ExternalOutput")
        dqt_d = nc.dram_tensor("dqt", (128, S), bf16, kind="ExternalOutput")
        dkt_d = nc.dram_tensor("dkt", (128, S), bf16, kind="ExternalOutput")
        dctx_d = nc.dram_tensor("dctx", (128, 2 * S), u8, kind="ExternalOutput")
        de_d = nc.dram_tensor("de", (128, 1024), u8, kind="ExternalOutput")
        dri_d = nc.dram_tensor("dri", (1, NQ), f32, kind="ExternalOutput")

    with tile.TileContext(nc) as tc:
        with (
            tc.tile_pool(name="persist", bufs=1) as pp,
            tc.tile_pool(name="epool", bufs=12) as epool,
            tc.tile_pool(name="rpool", bufs=3) as rpool,
            tc.tile_pool(name="opool", bufs=4) as opool,
            tc.tile_pool(name="psum", bufs=3, space="PSUM") as psum,
        ):
            # ---- PE warmup: dummy matmuls while DMAs land (HAM un-throttle) --
            dum = pp.tile([128, 640], f8, tag="dum")
            nc.gpsimd.memset(dum[:], 0.125)
            pw = psum.tile([128, 1024], f32, tag="big", bufs=3, name="psW")
            for i in range(16):
                nc.tensor.matmul(pw[:, 0:NQ], dum[:, 0:128], dum[:, 128:640],
                                 start=True, stop=True)

            # ---- load inputs ------------------------------------------------
            # weights + biases on the Activation HWDGE queue; x on the SP
            # queue in two phases (st01 first) so K-proj st0/st1 start early.
            # ~606ns issue cost per DMA makes few/big transfers essential.
            W = {}
            for nm, w_d in (("k", wk_d), ("q", wq_d), ("v", wv_d)):
                W[nm] = []
                for dp in range(DP):
                    t = pp.tile([128, 2 * EL], f8, tag=f"w{nm}{dp}", name=f"w{nm}{dp}")
                    W[nm].append(t)
            wo = pp.tile([128, 2 * D], f8, tag="wo")
            qbt = pp.tile([128, 2], f32, tag="qbt")
            kbt = pp.tile([128, 2], f32, tag="kbt")
            for nm, w_d in (("k", wk_d), ("q", wq_d)):
                for dp in range(DP):
                    nc.scalar.dma_start(W[nm][dp][:], w_d[dp * 128:(dp + 1) * 128, :])
            nc.scalar.dma_start(qbt[:], qb_d[:])
            nc.scalar.dma_start(kbt[:], kb_d[:])
            for dp in range(DP):
                nc.scalar.dma_start(W["v"][dp][:], wv_d[dp * 128:(dp + 1) * 128, :])
            nc.scalar.dma_start(wo[:], wo_d[:])

            xt = [pp.tile([128, 2 * S], f8, tag=f"xt{dp}", name=f"xt{dp}")
                  for dp in range(DP)]
            for sp in range(2):  # st-pairs: (st0,st1) then (st2,st3)
                for dp in range(DP):
                    for half in range(2):
                        lo = half * S + sp * 1024
                        nc.sync.dma_start(
                            xt[dp][:, lo:lo + 1024],
                            xt_d[dp * 128:(dp + 1) * 128, lo:lo + 1024],
                        )

            ebias = pp.tile([128, 1], f32, tag="ebias")
            nc.gpsimd.memset(ebias[:], -ECLAMP)

            # ---- persistent intermediates ----------------------------------
            QT = [pp.tile([128, S], bf16, tag=f"QT{c}", name=f"QT{c}") for c in range(2)]
            KT = [pp.tile([128, S], bf16, tag=f"KT{c}", name=f"KT{c}") for c in range(2)]
            Vp = [pp.tile([128, 2 * HL * VW], f8, tag=f"Vp{i}", name=f"Vp{i}")
                  for i in range(KP)]
            ctxT = pp.tile([128, 2 * S], f8, tag="ctxT")

            # ones columns of Vp (row-sum trick); junk pad cols 65-67 unread
            for kp in range(KP):
                v4 = Vp[kp][:, :].rearrange("p (k h c) -> p k h c", k=2, h=HL)
                nc.gpsimd.memset(v4[:, :, :, HD:HD + 1], 1.0)

            # ---- Q/K projections (fp8 DoubleRow, out bf16 with bias) --------
            def proj_qk(dstT, wt, bt, pc, st):
                ps = psum.tile([128, 1024], f32, tag="big", bufs=3, name="psP")
                lhs = []
                for dp in range(DP):
                    lhs.append(
                        wt[dp][:, pc * EL:(pc + 1) * EL].rearrange(
                            "p (k m) -> p k m", k=2
                        )
                    )
                rhs = [
                    xt[dp][:, :].rearrange("p (k s) -> p k s", k=2)[
                        :, :, st * NQ:(st + 1) * NQ
                    ]
                    for dp in range(DP)
                ]
                if drmode != "all":
                    for dc in range(2 * DP):
                        dp, half = dc // 2, dc % 2
                        nc.tensor.matmul(
                            ps[:, 0:NQ],
                            wt[dp][:, pc * EL + half * 128:
                                   pc * EL + half * 128 + 128],
                            xt[dp][:, half * S + st * NQ:
                                   half * S + (st + 1) * NQ],
                            start=(dc == 0), stop=(dc == 2 * DP - 1),
                        )
                else:
                    for dp in range(DP):
                        nc.tensor.matmul(
                            ps[:, 0:NQ], lhs[dp], rhs[dp],
                            start=(dp == 0), stop=(dp == DP - 1), perf_mode=DR,
                        )
                nc.scalar.activation(
                    dstT[pc][:, st * NQ:(st + 1) * NQ], ps[:, 0:NQ], IDENT,
                    bias=bt[:, pc:pc + 1], scale=1.0 / WQK_S,
                )

            for pc in range(2):
                for st in range(4):
                    proj_qk(KT, W["k"], kbt, pc, st)
            for pc in range(2):
                proj_qk(QT, W["q"], qbt, pc, 0)

            # ---- V projection (fp8 DR, packed into Vp with scale) -----------
            for sc in range(KC):
                ps = psum.tile([128, 1024], f32, tag="big", bufs=3, name="psV")
                for dp in range(DP):
                    nc.tensor.matmul(
                        ps[:, 0:EL],
                        xt[dp][:, :].rearrange("p (k s) -> p k s", k=2)[
                            :, :, sc * 128:(sc + 1) * 128
                        ],
                        W["v"][dp][:, :].rearrange("p (k e) -> p k e", k=2),
                        start=(dp == 0), stop=(dp == DP - 1), perf_mode=DR,
                    )
                kp, half = sc // 2, sc % 2
                v4 = Vp[kp][:, :].rearrange("p (k h c) -> p k h c", k=2, h=HL)
                nc.scalar.mul(
                    v4[:, half, :, 0:HD],
                    ps[:, 0:EL].rearrange("p (h c) -> p h c", c=HD),
                    sv,
                )

            # ---- attention --------------------------------------------------
            wo3 = wo[:, :].rearrange("p (k n) -> p k n", k=2)
            ctx3 = ctxT[:, :].rearrange("p (k s) -> p k s", k=2)
            po_n = 0

            def emit_outproj(oqt, mbs=(0, 1, 2, 3)):
                nonlocal po_n
                for mb in mbs:
                    m = oqt * 4 + mb
                    po = psum.tile([128, 1024], f32, tag="big", bufs=3, name="psO")
                    for n in range(2):
                        if drmode != "all":
                            for k in range(2):
                                nc.tensor.matmul(
                                    po[:, n * NQ:(n + 1) * NQ],
                                    ctxT[:, k * S + m * 128:k * S + (m + 1) * 128],
                                    wo[:, k * D + n * NQ:k * D + (n + 1) * NQ],
                                    start=(k == 0), stop=(k == 1),
                                )
                        else:
                            nc.tensor.matmul(
                                po[:, n * NQ:(n + 1) * NQ],
                                ctx3[:, :, m * 128:(m + 1) * 128],
                                wo3[:, :, n * NQ:(n + 1) * NQ],
                                start=True, stop=True, perf_mode=DR,
                            )
                    ot = opool.tile([128, 1024], bf16, tag="ot")
                    if po_n % 2 == 0:
                        nc.scalar.copy(ot[:], po[:])
                    else:
                        nc.vector.tensor_copy(ot[:], po[:])
                    po_n += 1
                    nc.sync.dma_start(out_d[m * 128:(m + 1) * 128, :], ot[:])

            for qt in range(4):
                for pc in range(2):
                    psc = [
                        psum.tile([65, NQ], f32, tag="psc", bufs=2, name=f"psc{par}")
                        for par in range(2)
                    ]
                    for kp in range(KP):
                        pss = [
                            psum.tile([128, 1024], f32, tag="big", bufs=3,
                                      name=f"psS{par}")
                            for par in range(2)
                        ]
                        # row-packed: alternate row groups so pairs overlap
                        for j in range(2):
                            kc = 2 * kp + j
                            for par in range(2):
                                nc.tensor.matmul(
                                    pss[par][:, j * NQ:(j + 1) * NQ],
                                    KT[pc][par * 64:(par + 1) * 64,
                                           kc * 128:(kc + 1) * 128],
                                    QT[pc][par * 64:(par + 1) * 64,
                                           qt * NQ:(qt + 1) * NQ],
                                    start=True, stop=True,
                                )
                        for par in range(2):
                            e = epool.tile([128, 1024], f8, tag="E")
                            if debug and qt == 0 and pc == 0 and kp == 0 and par == 0:
                                dbg_e = e
                            if _exp_engine(qt, pc, kp, par) == "act":
                                nc.scalar.activation(
                                    e[:], pss[par][:], EXP,
                                    scale=0.125, bias=ebias[:, 0:1],
                                )
                            else:
                                nc.vector.tensor_scalar(
                                    e[:].bitcast(u8), pss[par][:],
                                    SCHRAU_A, SCHRAU_B, MULT, ADD,
                                )
                            h = HL // 2 * pc + par  # head within group: 2*pc+par
                            if drmode == "none":
                                for j in range(2):
                                    nc.tensor.matmul(
                                        psc[par],
                                        Vp[kp][:, j * 272 + h * VW:
                                               j * 272 + h * VW + HD + 1],
                                        e[:, j * NQ:(j + 1) * NQ],
                                        start=(kp == 0 and j == 0),
                                        stop=(kp == KP - 1 and j == 1),
                                    )
                            else:
                                nc.tensor.matmul(
                                    psc[par],
                                    Vp[kp][:, :].rearrange("p (k c) -> p k c", k=2)[
                                        :, :, h * VW:h * VW + HD + 1
                                    ],
                                    e[:, :].rearrange("p (k n) -> p k n", k=2),
                                    start=(kp == 0), stop=(kp == KP - 1),
                                    perf_mode=DR,
                                )
                        # previous qt's out-projection, deferred and spread
                        # one m-block per odd kp so the PSUM ring never gets
                        # a 4-slot burst; next qt's Q projections interleave
                        # into pc1 -- both fill the per-kp PE idle that
                        # otherwise triggers HAM oscillation
                        if qt in (1, 2) and pc == 0 and kp in (1, 3, 5, 7):
                            emit_outproj(qt - 1, (kp // 2,))
                        if qt == 3 and kp in (1, 5):
                            emit_outproj(qt - 1, (2 * pc + (kp == 5),))
                        if qt < 3 and pc == 1 and kp in (2, 4):
                            proj_qk(QT, W["q"], qbt, kp // 2 - 1, qt + 1)
                    for par in range(2):
                        ri = rpool.tile([1, NQ], f32, tag="ri")
                        nc.vector.tensor_scalar(
                            ri[:].bitcast(i32), psc[par][64:65, :].bitcast(i32),
                            -1, RECIP_C, MULT, ADD,
                        )
                        if debug and qt == 0 and pc == 0 and par == 0:
                            nc.sync.dma_start(dri_d[:], ri[:])
                        rb = rpool.tile([64, NQ], f32, tag="rb")
                        nc.gpsimd.partition_broadcast(rb[:], ri[:])
                        nc.vector.tensor_tensor(
                            ctxT[par * 64:(par + 1) * 64,
                                 pc * S + qt * NQ:pc * S + (qt + 1) * NQ],
                            psc[par][0:HD, :], rb[:], MULT,
                        )
            emit_outproj(3)

            if debug:
                nc.sync.dma_start(dvp_d[:], Vp[0][:].bitcast(u8))
                nc.sync.dma_start(dqt_d[:], QT[0][:])
                nc.sync.dma_start(dkt_d[:], KT[0][:])
                nc.sync.dma_start(dctx_d[:], ctxT[:].bitcast(u8))
                nc.sync.dma_start(de_d[:], dbg_e[:].bitcast(u8))

    nc.compile()
    return nc


def _in_maps(inputs):
    x = np.asarray(inputs["x"], dtype=np.float32)
    alpha = float(np.asarray(inputs["alpha"]))
    q_w = np.asarray(inputs["q_w"], dtype=np.float32)
    k_w = np.asarray(inputs["k_w"], dtype=np.float32)
    v_w = np.asarray(inputs["v_w"], dtype=np.float32)
    o_w = np.asarray(inputs["o_w"], dtype=np.float32)
    q_b = np.asarray(inputs["q_b"], dtype=np.float32)
    k_b = np.asarray(inputs["k_b"], dtype=np.float32)

    fp8 = ml_dtypes.float8_e4m3fn

    def pack_x(xb):
        # [128*dp + p, half*S + s] = x[s, 128*(2dp+half)+p]
        xT = np.ascontiguousarray(xb.T)          # (1024, 2048)
        t = xT.reshape(DP, 2, 128, S).transpose(0, 2, 1, 3).reshape(DP * 128, 2 * S)
        return np.ascontiguousarray(t).astype(fp8)

    def pack_wqk(w, cols):
        # [128*dp + p, pc*256 + half*128 + e] = 32*w[128*(2dp+half)+p, 128*pc+e]
        ws = w[:, cols] * WQK_S                  # (1024, 256)
        t = ws.reshape(DP, 2, 128, 2, 128).transpose(0, 2, 3, 1, 4).reshape(
            DP * 128, 2 * EL)
        return np.ascontiguousarray(t).astype(fp8)

    def pack_wv(w, cols):
        # [128*dp + p, half*256 + e] = 32*w[128*(2dp+half)+p, e]
        ws = w[:, cols] * WV_S
        t = ws.reshape(DP, 2, 128, EL).transpose(0, 2, 1, 3).reshape(DP * 128, 2 * EL)
        return np.ascontiguousarray(t).astype(fp8)

    def pack_wo(w, rows):
        # [p, half*1024 + n] = 64*o_w[rowbase + 128*half + p, n]
        ws = w[rows, :] * WO_S                   # (256, 1024)
        t = ws.reshape(2, 128, D).transpose(1, 0, 2).reshape(128, 2 * D)
        return np.ascontiguousarray(t).astype(fp8)

    xp = [pack_x(x[b]) for b in range(B)]
    maps = []
    for c in range(8):
        b, hp = divmod(c, 4)
        cols = slice(EL * hp, EL * (hp + 1))
        maps.append(
            {
                "xt": xp[b],
                "wq": pack_wqk(q_w, cols),
                "wk": pack_wqk(k_w, cols),
                "wv": pack_wv(v_w, cols),
                "wo": pack_wo(o_w, cols),
                "qb": np.ascontiguousarray(q_b[cols].reshape(2, 128).T),
                "kb": np.ascontiguousarray(k_b[cols].reshape(2, 128).T),
            }
        )
    return maps, alpha


LAST_RESULT = None


def kernel(**inputs):
    global LAST_RESULT
    _install_ntff_hook_shim()
    maps, alpha = _in_maps(inputs)
    key = round(alpha, 12)
    if key not in _CACHE:
        _CACHE[key] = _build(alpha)
    nc = _CACHE[key]
    res = bass_utils.run_bass_kernel_spmd(nc, maps, core_ids=list(range(8)))
    LAST_RESULT = res

    x = np.asarray(inputs["x"], dtype=np.float64)
    ds = np.asarray(inputs["docking_scores"], dtype=np.float64)
    v_w = np.asarray(inputs["v_w"], dtype=np.float64)
    o_w = np.asarray(inputs["o_w"], dtype=np.float64)
    v_b = np.asarray(inputs["v_b"], dtype=np.float64)
    o_b = np.asarray(inputs["o_b"], dtype=np.float64)

    out = np.empty((B, S, D), dtype=np.float32)
    for b in range(B):
        dev = np.zeros((S, D), dtype=np.float32)
        for c in range(4 * b, 4 * b + 4):
            dev += res.results[c]["out"].astype(np.float32)
        dev /= OUT_DIV
        # exact rank-1 docking path (+ V bias via attention row-sum)
        dsx = ds[b] @ x[b]                                   # (1024,)
        cvec = alpha * (dsx @ v_w) + ((1.0 - alpha) + alpha * ds[b].sum()) * v_b
        row = cvec @ o_w + o_b                               # (1024,)
        out[b] = dev + row[None, :].astype(np.float32)
    return out
